# revision 74
# baseline (speedup 1.0000x reference)
"""Trainium2 distributed Bass kernel for nn_Actor (gnn_message_passing).

Strategy (column-parallel 1D partition, transposed layout):
  Each of the 8 cores owns a 512-column slice B_c of the [4096,4096] outputs.
  All heavy tensors live TRANSPOSED on device ([node-b on partitions]), so the
  per-column persona weights become per-partition scalars and the row-norm
  broadcasts fall out of ones-matmuls.

  1. Each core computes its own EAT columns EAT[:, B_c] = (edges@attr).T[:,B_c]
     fully locally (full-k contraction with edges rows B_c), plus EA rows B_c
     in natural layout (f32). ONE small AllGather (256 KB/rank) distributes
     all EAT columns to every core — no AllReduce needed anywhere.
  2. Per (half, i): x_i = attrT + kappa_i*EAT (kappa=(1-r)W/r; cosine is
     scale-invariant), inv-norms via ones-matmul colsum + Ln/Exp
     ( 1/sqrt(x) = exp(-0.5 ln x) ), nf1_i = x_i * binv (bf16).
  3. Main loop over [128b x 1024a] tiles: sim tile via matmul; the disjoint
     create/delete masks collapse into ONE exp + ONE tanh:
        Z = union_mask * e_i * exp(invT_i*(s - edge_mask*s^2)),  W = tanh(Z)
     accumulated into PSUM with a diag(persona) matmul per i.
  4. Small [N,D] outputs computed in natural layout from the locally-computed
     f32 EA rows.
"""
import sys
import numpy as np

sys.path.insert(0, '/opt/trn_rl_repo')

import ml_dtypes

import concourse.bass as bass
import concourse.bacc as bacc
import concourse.mybir as mybir
from concourse.tile import TileContext
from concourse.bass_utils import run_bass_kernel_spmd

N, D, K, NC = 4096, 256, 4, 8
B = N // NC            # 512 local columns per core
NBT = B // 128         # 4 partition blocks of local columns
SA = 1024              # a-strip size in main loop
NST = N // SA          # 4 strips
bf16 = ml_dtypes.bfloat16
AF = mybir.ActivationFunctionType
ALU = mybir.AluOpType
f32 = mybir.dt.float32
bf = mybir.dt.bfloat16

_BUILD_CACHE = {}


def _build(scalars):
    invT, lne, kap, cc, rr = scalars
    nc = bacc.Bacc(num_devices=NC)

    def register_const(value, dtype=f32):
        t = nc.alloc_sbuf_tensor(f"uconst-{dtype.name}-{value}", [128, 1], dtype)
        nc.gpsimd.memset(t.ap(), value)
        nc.const_aps.aps[(dtype, value)] = t.ap()

    for v in set(list(lne) + [1e-10]):
        register_const(float(v))
    nc.all_engine_barrier()

    # ---- I/O ----
    et_d = nc.declare_dram_parameter("et", [128, 4, N], bf, isOutput=False)
    tth_d = nc.declare_dram_parameter("tth", [128, 4, N], bf, isOutput=False)
    attrT_d = nc.declare_dram_parameter("attrT", [128, 2, N], bf, isOutput=False)
    attrTl_d = nc.declare_dram_parameter("attrTl", [128, 2, B], bf, isOutput=False)
    attrn_d = nc.declare_dram_parameter("attrn", [128, 4, D], f32, isOutput=False)
    dg_d = nc.declare_dram_parameter("dg", [128, 16 * 128], bf, isOutput=False)
    ones_d = nc.declare_dram_parameter("ones", [128, 128], bf, isOutput=False)
    erT_d = nc.declare_dram_parameter("erT", [128, 32, B], bf, isOutput=False)
    attrf_d = nc.declare_dram_parameter("attrf", [128, 32, D], bf, isOutput=False)

    pT_d = nc.declare_dram_parameter("pT", [128, 4, N], bf, isOutput=True)
    aprob_d = nc.declare_dram_parameter("aprob", [128, 4, D], f32, isOutput=True)
    fsig_d = nc.declare_dram_parameter("fsig", [128, 4, D], f32, isOutput=True)
    nfeat_d = nc.declare_dram_parameter("nfeat", [128, 4, D], f32, isOutput=True)
    sattr_d = nc.declare_dram_parameter("sattr", [128, 4, D], f32, isOutput=True)
    sneigh_d = nc.declare_dram_parameter("sneigh", [128, 4, D], f32, isOutput=True)

    agin_d = nc.dram_tensor("agin", [128, 2, B], bf)
    agout_d = nc.dram_tensor("agout", [NC * 128, 2, B], bf,
                             addr_space="Shared")

    groups = [list(range(NC))]

    with TileContext(nc) as tc:
        with tc.tile_pool(name="res", bufs=1) as res:
            # resident tensors (et loaded later; not needed until main loop)
            et_s = res.tile([128, 4, N], bf)
            dg_s = res.tile([128, 16 * 128], bf)
            nc.sync.dma_start(out=dg_s[:], in_=dg_d[:])
            ones_s = res.tile([128, 128], bf)
            nc.sync.dma_start(out=ones_s[:], in_=ones_d[:])
            attrn_s = res.tile([128, 4, D], f32)
            nc.sync.dma_start(out=attrn_s[:], in_=attrn_d[:])
            attrTl_s = res.tile([128, 2, B], bf)
            nc.sync.dma_start(out=attrTl_s[:], in_=attrTl_d[:])
            eanat_s = res.tile([128, 4, D], f32)
            eatloc_s = res.tile([128, 2, B], bf)
            nf1h_s = [res.tile([128, 2 * K, N // 2], bf, name=f"nf1h{h}")
                      for h in range(2)]             # 2 x 4 MB
            nf1l_s = res.tile([128, 2 * K, B], bf)    # 1 MB

            # ---- EAT columns B_c computed fully locally (full-k), then
            # a single small AllGather distributes all columns to all cores --
            early = tc.tile_pool(name="early", bufs=1)
            pe_pool = early.__enter__()
            erT_s = pe_pool.tile([128, 32, B], bf)
            attrf_s = pe_pool.tile([128, 32, D], bf)
            for g in range(4):
                nc.sync.dma_start(out=erT_s[:, g * 8:(g + 1) * 8, :],
                                  in_=erT_d[:, g * 8:(g + 1) * 8, :])
                nc.sync.dma_start(out=attrf_s[:, g * 8:(g + 1) * 8, :],
                                  in_=attrf_d[:, g * 8:(g + 1) * 8, :])
            nc.sync.dma_start(out=et_s[:], in_=et_d[:])
            with tc.tile_pool(name="phB", bufs=2) as pb, \
                 tc.tile_pool(name="phBp", bufs=2, space="PSUM") as pbp:
                for dt in range(2):
                    ps = pbp.tile([128, B], f32, tag="eatps")
                    for kt in range(32):
                        nc.tensor.matmul(
                            ps[:],
                            attrf_s[:, kt, dt * 128:(dt + 1) * 128],
                            erT_s[:, kt, :],
                            start=(kt == 0), stop=(kt == 31))
                    nc.vector.tensor_copy(eatloc_s[:, dt, :], ps[:])
                    tb = pb.tile([128, B], bf, tag="eatsb")
                    nc.vector.tensor_copy(tb[:], ps[:])
                    nc.sync.dma_start(out=agin_d[:, dt, :], in_=tb[:])
                nc.gpsimd.collective_compute(
                    "AllGather", ALU.bypass, replica_groups=groups,
                    ins=[agin_d[:]], outs=[agout_d[:]])

            # ---- work that overlaps the AllReduces ----
            # EA natural rows for B_c, computed locally in f32
            with tc.tile_pool(name="phN", bufs=1) as pn, \
                 tc.tile_pool(name="phNp", bufs=2, space="PSUM") as pnp, \
                 tc.tile_pool(name="phN2", bufs=2) as pn2:
                for mt in range(4):
                    psn = pnp.tile([128, D], f32, tag="natps")
                    for kt in range(32):
                        nc.tensor.matmul(
                            psn[:],
                            erT_s[:, kt, mt * 128:(mt + 1) * 128],
                            attrf_s[:, kt, :],
                            start=(kt == 0), stop=(kt == 31))
                    nc.vector.tensor_copy(eanat_s[:, mt, :], psn[:])


            early.__exit__(None, None, None)

            # ---- phase 3: [N,D] outputs in natural layout ----
            with tc.tile_pool(name="p3", bufs=2) as p3, \
                 tc.tile_pool(name="p3p", bufs=1, space="PSUM") as p3p:
                apPs = [p3p.tile([128, D], f32, tag=f"apP{t}", name=f"apP{t}")
                        for t in range(4)]
                for i in range(K):
                    rattr = p3.tile([128, 4, D], f32, tag="rattr")
                    nc.vector.tensor_scalar(rattr[:], attrn_s[:],
                                            float(rr[i]), None, op0=ALU.mult)
                    nfv = p3.tile([128, 4, D], f32, tag="nfv")
                    nc.vector.scalar_tensor_tensor(
                        nfv[:], eanat_s[:], float(cc[i]), rattr[:],
                        op0=ALU.mult, op1=ALU.add)
                    th = p3.tile([128, 4, D], f32, tag="th")
                    nc.scalar.activation(th[:], nfv[:], AF.Tanh, scale=0.5)
                    sgb = p3.tile([128, 4, D], bf, tag="sgb")
                    nc.vector.tensor_scalar(sgb[:], th[:], 0.5, 0.5,
                                            op0=ALU.mult, op1=ALU.add)
                    for t in range(4):
                        nc.tensor.matmul(
                            apPs[t][:],
                            dg_s[:, (t * 4 + i) * 128:(t * 4 + i + 1) * 128],
                            sgb[:, t, :],
                            start=(i == 0), stop=(i == K - 1))
                    if i == K - 1:
                        nc.sync.dma_start(out=nfeat_d[:], in_=nfv[:])
                        fsg = p3.tile([128, 4, D], f32, tag="fsg")
                        nc.vector.tensor_scalar(fsg[:], th[:], 0.5, 0.5,
                                                op0=ALU.mult, op1=ALU.add)
                        nc.sync.dma_start(out=fsig_d[:], in_=fsg[:])
                        nc.sync.dma_start(out=sattr_d[:], in_=rattr[:])
                        sng = p3.tile([128, 4, D], f32, tag="sng")
                        nc.vector.tensor_scalar(sng[:], eanat_s[:],
                                                float(cc[i]), None,
                                                op0=ALU.mult)
                        nc.sync.dma_start(out=sneigh_d[:], in_=sng[:])
                apo = p3.tile([128, 4, D], f32, tag="apo")
                for t in range(4):
                    nc.scalar.activation(apo[:, t, :], apPs[t][:], AF.Copy)
                nc.sync.dma_start(out=aprob_d[:], in_=apo[:])



            # local-columns nf1l for all i
            with tc.tile_pool(name="lp", bufs=2) as lp, \
                 tc.tile_pool(name="lpp", bufs=2, space="PSUM") as lpp:
                for i in range(K):
                    xql = lp.tile([128, 2, B], bf, tag="xql")
                    for dt in range(2):
                        nc.vector.scalar_tensor_tensor(
                            xql[:, dt, :], eatloc_s[:, dt, :], float(kap[i]),
                            attrTl_s[:, dt, :], op0=ALU.mult, op1=ALU.add)
                    pcl = lpp.tile([128, B], f32, tag="colpsl")
                    for dt in range(2):
                        sql = lp.tile([128, B], bf, tag="sql")
                        nc.vector.tensor_tensor(
                            sql[:], xql[:, dt, :], xql[:, dt, :], op=ALU.mult)
                        nc.tensor.matmul(pcl[:], ones_s[:], sql[:],
                                         start=(dt == 0), stop=(dt == 1))
                    nc.scalar.activation(pcl[:], pcl[:], AF.Ln, bias=1e-10)
                    binvl = lp.tile([128, B], bf, tag="binvl")
                    nc.scalar.activation(binvl[:], pcl[:], AF.Exp,
                                         bias=0.0, scale=-0.5)
                    for dt in range(2):
                        nc.vector.tensor_tensor(
                            nf1l_s[:, 2 * i + dt, :], xql[:, dt, :], binvl[:],
                            op=ALU.mult)

            # ---- per-(half, i) prep: nf1 (full, transposed) ----
            with tc.tile_pool(name="prep", bufs=1) as pr, \
                 tc.tile_pool(name="prep2", bufs=3) as pr2, \
                 tc.tile_pool(name="prpp", bufs=2, space="PSUM") as prp:
                attrT_s = pr.tile([128, 2, N], bf)
                nc.sync.dma_start(out=attrT_s[:], in_=attrT_d[:])
                eat_s = pr.tile([128, 2, N], bf)
                for c in range(NC):
                    for dt in range(2):
                        nc.sync.dma_start(
                            out=eat_s[:, dt, c * B:(c + 1) * B],
                            in_=agout_d[c * 128:(c + 1) * 128, dt, :])
                H = N // 2
                for h in range(2):
                    for i in range(K):
                        xqh = pr2.tile([128, 2, H], bf, tag="xqh")
                        for dt in range(2):
                            nc.vector.scalar_tensor_tensor(
                                xqh[:, dt, :], eat_s[:, dt, h * H:(h + 1) * H],
                                float(kap[i]),
                                attrT_s[:, dt, h * H:(h + 1) * H],
                                op0=ALU.mult, op1=ALU.add)
                        pc = prp.tile([128, H], f32, tag="colps")
                        for dt in range(2):
                            sq = pr2.tile([128, H], bf, tag="sq")
                            if dt == 0:
                                nc.scalar.activation(sq[:], xqh[:, dt, :],
                                                     AF.Square)
                            else:
                                nc.vector.tensor_tensor(
                                    sq[:], xqh[:, dt, :], xqh[:, dt, :],
                                    op=ALU.mult)
                            for nt in range(4):
                                nc.tensor.matmul(
                                    pc[:, nt * 512:(nt + 1) * 512], ones_s[:],
                                    sq[:, nt * 512:(nt + 1) * 512],
                                    start=(dt == 0), stop=(dt == 1))
                        nc.scalar.activation(pc[:], pc[:], AF.Ln, bias=1e-10)
                        binvh = pr2.tile([128, H], bf, tag="binvh")
                        nc.scalar.activation(binvh[:], pc[:],
                                             AF.Exp, bias=0.0, scale=-0.5)
                        for dt in range(2):
                            nc.vector.tensor_tensor(
                                nf1h_s[h][:, 2 * i + dt, :],
                                xqh[:, dt, :], binvh[:], op=ALU.mult)

            # ---- main loop ----
            with tc.tile_pool(name="mn", bufs=4) as mn, \
                 tc.tile_pool(name="mnS", bufs=2, space="PSUM") as mnS, \
                 tc.tile_pool(name="mnP", bufs=2, space="PSUM") as mnP:
                for st in range(NST):
                    for bt in range(NBT):
                        a0 = st * SA
                        dm = mn.tile([128, SA], bf, tag="dm")
                        nc.sync.dma_start(out=dm[:],
                                          in_=tth_d[:, bt, a0:a0 + SA])
                        nc.vector.tensor_scalar(dm[:], dm[:], 1.0, None,
                                                op0=ALU.min)
                        nc.vector.tensor_tensor(
                            dm[:], dm[:], et_s[:, bt, a0:a0 + SA], op=ALU.max)
                        pP = mnP.tile([128, SA], f32, tag="pP")
                        for i in range(K):
                            pS = mnS.tile([128, SA], f32, tag="pS")
                            hh = a0 // (N // 2)
                            ao = a0 % (N // 2)
                            for nt in range(2):
                                for dt in range(2):
                                    nc.tensor.matmul(
                                        pS[:, nt * 512:(nt + 1) * 512],
                                        nf1l_s[:, 2 * i + dt,
                                               bt * 128:(bt + 1) * 128],
                                        nf1h_s[hh][:, 2 * i + dt,
                                                   ao + nt * 512:
                                                   ao + (nt + 1) * 512],
                                        start=(dt == 0), stop=(dt == 1))
                            s2 = mn.tile([128, SA], bf, tag="s2")
                            nc.scalar.activation(s2[:], pS[:], AF.Square)
                            ms2 = mn.tile([128, SA], bf, tag="ms2")
                            nc.vector.tensor_tensor(
                                ms2[:], et_s[:, bt, a0:a0 + SA], s2[:],
                                op=ALU.mult)
                            argn = mn.tile([128, SA], bf, tag="argn")
                            nc.vector.tensor_tensor(argn[:], pS[:], ms2[:],
                                                    op=ALU.subtract)
                            Et = mn.tile([128, SA], bf, tag="Et")
                            nc.scalar.activation(Et[:], argn[:], AF.Exp,
                                                 bias=float(lne[i]),
                                                 scale=float(invT[i]))
                            Zt = mn.tile([128, SA], bf, tag="Zt")
                            nc.vector.tensor_tensor(Zt[:], dm[:], Et[:],
                                                    op=ALU.mult)
                            Wt = mn.tile([128, SA], bf, tag="Wt")
                            nc.scalar.activation(Wt[:], Zt[:], AF.Tanh)
                            for nt in range(2):
                                nc.tensor.matmul(
                                    pP[:, nt * 512:(nt + 1) * 512],
                                    dg_s[:, (bt * 4 + i) * 128:
                                         (bt * 4 + i + 1) * 128],
                                    Wt[:, nt * 512:(nt + 1) * 512],
                                    start=(i == 0), stop=(i == K - 1))
                        po = mn.tile([128, SA], bf, tag="po", bufs=1)
                        nc.vector.tensor_copy(po[:], pP[:])
                        nc.sync.dma_start(out=pT_d[:, bt, a0:a0 + SA],
                                          in_=po[:])

    nc.finalize()
    return nc


def _part_major(x, nblk):
    """[nblk*128, F] -> [128, nblk, F]"""
    F = x.shape[1]
    return np.ascontiguousarray(
        x.reshape(nblk, 128, F).transpose(1, 0, 2))


def _prepare(attributes, edges, two_hop, persona, T, e, r, W, times):
    attributes = np.asarray(attributes, dtype=np.float32)
    edges = np.asarray(edges, dtype=np.float32)
    two_hop = np.asarray(two_hop, dtype=np.float32)
    persona = np.asarray(persona, dtype=np.float32)
    T = np.asarray(T, dtype=np.float32)
    e = np.asarray(e, dtype=np.float32)
    r = np.asarray(r, dtype=np.float32)
    W = np.asarray(W, dtype=np.float32)
    ti = int(times)

    invT = (1.0 / T.astype(np.float64)).astype(np.float64)
    lne = np.log(e.astype(np.float64))
    rr = r.astype(np.float64)
    cc = ((1.0 - rr) * W.astype(np.float64))
    kap = cc / rr

    key = tuple(np.concatenate([invT, lne, kap, cc, rr]).tolist())
    if key not in _BUILD_CACHE:
        _BUILD_CACHE[key] = _build((invT, lne, kap, cc, rr))
    nc = _BUILD_CACHE[key]

    eT = edges.T
    tT = two_hop.T
    aT = attributes.T  # [D, N]
    pv_all = persona[ti]  # [N, K]

    ones = np.ones((128, 128), dtype=bf16)

    in_maps = []
    for c in range(NC):
        sl = slice(c * B, (c + 1) * B)
        pvl = pv_all[sl]  # [512, K]
        dg = np.zeros((128, 16, 128), dtype=bf16)
        for t in range(4):
            for i in range(K):
                blk = pvl[t * 128:(t + 1) * 128, i].astype(bf16)
                np.fill_diagonal(dg[:, t * 4 + i, :], blk)
        in_maps.append({
            "et": _part_major(eT[sl], 4).astype(bf16),
            "tth": _part_major(tT[sl], 4).astype(bf16),
            "attrT": _part_major(aT, 2).astype(bf16),
            "attrTl": _part_major(aT[:, sl], 2).astype(bf16),
            "attrn": _part_major(attributes[sl], 4).astype(np.float32),
            "dg": np.ascontiguousarray(dg.reshape(128, 16 * 128)),
            "ones": ones,
            "erT": _part_major(np.ascontiguousarray(edges[sl].T), 32).astype(bf16),
            "attrf": _part_major(attributes, 32).astype(bf16),
        })

    return nc, in_maps


def _gather(results):
    edges_prob = np.empty((N, N), dtype=np.float32)
    attr_prob = np.empty((N, D), dtype=np.float32)
    feat_sig = np.empty((N, D), dtype=np.float32)
    next_feat = np.empty((N, D), dtype=np.float32)
    sattr = np.empty((N, D), dtype=np.float32)
    sneigh = np.empty((N, D), dtype=np.float32)

    def _unpart(x):
        """[128, nblk, F] -> [nblk*128, F]"""
        return x.transpose(1, 0, 2).reshape(-1, x.shape[2])

    for c in range(NC):
        sl = slice(c * B, (c + 1) * B)
        rc = results[c]
        pTc = _unpart(np.asarray(rc["pT"]).astype(np.float32))  # [512, 4096]
        edges_prob[:, sl] = pTc.T
        attr_prob[sl] = _unpart(np.asarray(rc["aprob"]))
        feat_sig[sl] = _unpart(np.asarray(rc["fsig"]))
        next_feat[sl] = _unpart(np.asarray(rc["nfeat"]))
        sattr[sl] = _unpart(np.asarray(rc["sattr"]))
        sneigh[sl] = _unpart(np.asarray(rc["sneigh"]))

    return (edges_prob, attr_prob, feat_sig, next_feat, sattr, sneigh)


def kernel(attributes, edges, two_hop, persona, T, e, r, W, times,
           _trace=False):
    nc, in_maps = _prepare(attributes, edges, two_hop, persona,
                           T, e, r, W, times)
    res = run_bass_kernel_spmd(nc, in_maps, core_ids=list(range(NC)))
    out = _gather(res.results)
    if _trace:
        return out, res
    return out
